# revision 30
# baseline (speedup 1.0000x reference)
"""Trainium2 Bass kernel for the 8-layer dilated tanh-RNN stack.

Reductions used:

1) Cone reduction (exact): the model output is `h_last @ W + b` where h_last
   is the last timestep of the last layer only.  Recursively, each dilated
   layer only needs ONE of its dilation lanes, so each layer is a plain
   32-wide (batch-sharded) tanh-RNN over a subsampled sequence:
   784+392+196+98+49+25+13+7 = 1564 sequential cell steps per core.
   (Verified exact vs reference: rel err 3e-7.)

2) Chunked scan with warmup (approximate, error well below bf16 noise): the
   recurrence is contractive (|Whh| spectral radius ~0.57), so state forgets
   initial conditions at ~0.5^k.  Layers 0-3 are split into C=7 time chunks
   run in parallel as extra batch columns; each chunk (except the first)
   starts W=8 steps early from zero state to converge to the true state.
   Serial depth: L0 784 -> 120 steps of width 224.  Chunk 0's state is
   re-zeroed at the warmup/real boundary so it remains exact.

3) bf16 weights/states (PSUM accumulation, biases and classifier head stay
   fp32): measured rel err vs fp32 reference ~4e-3.

Sharding: data-parallel over batch, 32 batch elements per core, weights
replicated, no collectives.  Host packs weights transposed (lhsT) and x
time-major.
"""

import numpy as np

_CACHE: dict = {}

N_STEPS = [784, 392, 196, 98, 49, 25, 13, 7]
C = 7            # chunks for layers 0-3
W = 8            # warmup steps
M = [112, 56, 28, 14]  # chunk length (own steps) for layers 0-3


def _ensure_path():
    import sys
    for p in ("/opt/trn_rl_repo",):
        if p not in sys.path:
            sys.path.insert(0, p)


def _build_nc(reps=1, variant="base", loop_reps=1):
    _ensure_path()
    import contextlib
    import concourse.bacc as bacc
    import concourse.bass as bass
    import concourse.mybir as mybir
    import concourse.tile as tile

    f32 = mybir.dt.float32
    bf16 = mybir.dt.bfloat16
    Tanh = mybir.ActivationFunctionType.Tanh

    nc = bacc.Bacc("TRN2", target_bir_lowering=False, debug=False, num_devices=8)

    xrow_d = nc.declare_dram_parameter("xrow", (1, 25088), bf16, isOutput=False)
    wpack_d = nc.declare_dram_parameter("wpack", (128, 1920), bf16, isOutput=False)
    fpack_d = nc.declare_dram_parameter("fpack", (128, 18), f32, isOutput=False)
    srow_d = nc.declare_dram_parameter("srow", (1, 128), bf16, isOutput=False)
    frow_d = nc.declare_dram_parameter("frow", (1, 42), f32, isOutput=False)
    out_d = nc.declare_dram_parameter("out", (32, 10), f32, isOutput=True)

    with tile.TileContext(nc) as tc:
        with (
            tc.tile_pool(name="sb", bufs=1) as sb,
            tc.tile_pool(name="ps", bufs=1, space=bass.MemorySpace.PSUM) as ps,
        ):
            wpack = sb.tile([128, 1920], bf16)
            fpack = sb.tile([128, 18], f32)
            srow = sb.tile([1, 128], bf16)
            frow = sb.tile([1, 42], f32)
            # x padded row: W zero slots | 784 data slots | M[0] tail pad
            xpad = sb.tile([1, (W + 784 + M[0]) * 32], bf16)
            # chunked-consumer input buffers: W zero prefix + n slots + tail pad
            Ibuf = [None,
                    sb.tile([128, (W + 392 + M[1]) * 32], bf16, name="I1"),
                    sb.tile([128, (W + 196 + M[2]) * 32], bf16, name="I2"),
                    sb.tile([128, (W + 98 + M[3]) * 32], bf16, name="I3"),
                    sb.tile([128, (49 + C) * 32], bf16, name="I4"),
                    sb.tile([128, 512], bf16, name="I5"),
                    sb.tile([128, 512], bf16, name="I6"),
                    sb.tile([128, 512], bf16, name="I7")]
            S = ([[sb.tile([128, 224], bf16, name=f"cs{l}_{i}") for i in range(2)]
                  for l in range(4)]
                 + [sb.tile([128, 32], bf16, name=f"s{l}") for l in range(4, 8)])
            H = sb.tile([128, 32], f32)
            out_sb = sb.tile([32, 10], f32)

            # zero prefixes (read by chunk-0 warmups)
            nc.gpsimd.memset(xpad[:, 0:W * 32], 0.0)
            for l in (1, 2, 3):
                nc.gpsimd.memset(Ibuf[l][:, 0:W * 32], 0.0)

            # spread input DMAs across DGE queues so they land in parallel
            # (chunked L0's first step reads ALL of x, so startup latency is
            # gated by the slowest of these)
            nc.sync.dma_start(wpack[:], wpack_d[:])
            nc.scalar.dma_start(fpack[:], fpack_d[:])
            nc.scalar.dma_start(srow[:], srow_d[:])
            nc.scalar.dma_start(frow[:], frow_d[:])
            qs = [nc.sync, nc.scalar, nc.gpsimd]
            for c in range(8):
                qs[c % 3].dma_start(
                    xpad[:, W * 32 + c * 3136: W * 32 + (c + 1) * 3136],
                    xrow_d[:, c * 3136:(c + 1) * 3136])

            whhT = [wpack[:, l * 128:(l + 1) * 128] for l in range(8)]
            wihT = [None] + [wpack[:, 1024 + (l - 1) * 128: 1024 + l * 128]
                             for l in range(1, 8)]
            bias = [fpack[:, l: l + 1] for l in range(8)]
            wcls = fpack[:, 8:18]
            wih0T = srow[:, 0:128]
            bcls = frow[:, 0:10]
            ones32 = frow[:, 10:42]

            # PSUM: L0/L1 two banks each (4-deep slice rotation -- more WAR
            # slack on the hottest chains), L2/L3 one bank each (2-deep),
            # L4 one bank (16-slot), shared bank for L5-7 + head. Total = 8.
            cbank = [[ps.tile([128, 512], f32, name=f"cb{l}_{i}")
                      for i in range(2)] for l in range(2)] + \
                    [ps.tile([128, 512], f32, name=f"cb{l}") for l in (2, 3)]
            p4 = ps.tile([128, 512], f32, name="p4")
            shared = ps.tile([128, 512], f32, name="p567")

            def strided(buf, start_col, stride_cols):
                """[128, C, 32] AP over `buf`: cols start + k*stride for k<C."""
                sl = buf[:, start_col: start_col + C * stride_cols]
                return sl.rearrange("p (c w) -> p c w", c=C)[:, :, 0:32]

            def xstrided(start_col, stride_cols):
                sl = xpad[:, start_col: start_col + C * stride_cols]
                return sl.rearrange("p (c w) -> p c w", c=C)[:, :, 0:32]

            def psum_slice(l, s):
                if l <= 1:
                    b = cbank[l][(s % 4) // 2]
                    return b[:, (s % 2) * 256:(s % 2) * 256 + 224]
                if l <= 3:
                    return cbank[l][:, (s % 2) * 256:(s % 2) * 256 + 224]
                if l == 4:
                    return p4[:, (s % 16) * 32:(s % 16) * 32 + 32]
                if l == 5:
                    return shared[:, (s % 8) * 32:(s % 8) * 32 + 32]
                if l == 6:
                    return shared[:, 256 + (s % 4) * 32: 256 + (s % 4) * 32 + 32]
                return shared[:, 384 + (s % 2) * 32: 384 + (s % 2) * 32 + 32]

            def act_target(l, j):
                """Output AP for layer l at global step j (chunked l<=3)."""
                if l <= 3:
                    if j < W or (j - W) % 2 == 0:
                        return S[l][j % 2][:]
                    q = (j - W - 1) // 2
                    if l < 3:
                        # into Ibuf[l+1]: slot q_k = k*(M[l]/2)+q, +W prefix
                        return strided(Ibuf[l + 1], (q + W) * 32,
                                       (M[l] // 2) * 32)
                    # l == 3 -> I4 (unchunked consumer, no prefix)
                    return strided(Ibuf[4], q * 32, (M[3] // 2) * 32)
                if l <= 6:
                    if j % 2 == 0:
                        k = (j // 2) % 16
                        return Ibuf[l + 1][:, k * 32:k * 32 + 32]
                    return S[l][:]
                return H[:] if j == 6 else S[7][:]

            Tanh_ = Tanh

            def emit_act(tgt, sl, l):
                if variant == "dvecopy":
                    nc.vector.tensor_copy(tgt, sl)
                else:
                    nc.scalar.activation(tgt, sl, Tanh_, bias=bias[l])

            n_glob = [M[0] + W, M[1] + W, M[2] + W, M[3] + W, 49, 25, 13, 7]

            def can_run(l, j, progress):
                if j >= n_glob[l]:
                    return False
                if l == 0:
                    return True
                if l <= 3:
                    need = 0
                    for k in range(C):
                        s_k = k * (N_STEPS[l] // C) - W + j
                        if s_k < 0:
                            continue
                        t = 2 * s_k + 1
                        kp, loc = divmod(t, M[l - 1])
                        need = max(need, W + loc + 1)
                    return progress[l - 1] >= need
                if l == 4:
                    t = 2 * j + 1
                    kp, loc = divmod(t, M[3])
                    return progress[3] >= W + loc + 1
                return progress[l - 1] >= 2 * j + 1

            def emit_step(l, j, prev):
                if l <= 3:
                    sl = psum_slice(l, j)
                    if l == 0:
                        nc.tensor.matmul(sl, wih0T, xstrided(j * 32, M[0] * 32),
                                         start=True, stop=(j == 0),
                                         skip_group_check=True)
                    else:
                        rhs = strided(Ibuf[l], j * 32, (N_STEPS[l] // C) * 32)
                        nc.tensor.matmul(sl, wihT[l], rhs,
                                         start=True, stop=(j == 0),
                                         skip_group_check=True)
                    if j > 0:
                        nc.tensor.matmul(sl, whhT[l], prev[l],
                                         start=False, stop=True,
                                         skip_group_check=True)
                    tgt = act_target(l, j)
                    emit_act(tgt, sl, l)
                    prev[l] = tgt
                    if j == W - 1:
                        # chunk 0 enters its real region from exact zero state
                        # (tgt at j=W-1 is S[l][(W-1)%2]; zero chunk 0's cols)
                        nc.gpsimd.memset(S[l][(W - 1) % 2][:, 0:32], 0.0)
                else:
                    sl = psum_slice(l, j)
                    k = j % 16
                    nc.tensor.matmul(sl, wihT[l], Ibuf[l][:, j * 32:(j + 1) * 32]
                                     if l == 4 else Ibuf[l][:, k * 32:k * 32 + 32],
                                     start=True, stop=(j == 0),
                                     skip_group_check=True)
                    if j > 0:
                        nc.tensor.matmul(sl, whhT[l], prev[l],
                                         start=False, stop=True,
                                         skip_group_check=True)
                    tgt = act_target(l, j)
                    emit_act(tgt, sl, l)
                    prev[l] = tgt

            def emit_cone():
                prev = [None] * 8
                progress = [0] * 8
                total = sum(n_glob)
                done = 0
                while done < total:
                    stuck = True
                    for l in range(8):
                        if can_run(l, progress[l], progress):
                            emit_step(l, progress[l], prev)
                            progress[l] += 1
                            done += 1
                            stuck = False
                    assert not stuck, f"schedule deadlock at {progress}"
                # classifier head
                head = shared[0:32, 448:458]
                nc.tensor.matmul(head, H[:], wcls, start=True, stop=False,
                                 skip_group_check=True)
                nc.tensor.matmul(head, ones32, bcls, start=False, stop=True,
                                 skip_group_check=True)
                nc.vector.tensor_copy(out_sb[:], head)

            loop_ctx = (tc.For_i(0, loop_reps, 1) if loop_reps > 1
                        else contextlib.nullcontext())
            with loop_ctx:
                for _rep in range(reps):
                    emit_cone()

            nc.sync.dma_start(out_d[:], out_sb[:])

    nc.compile()
    return nc


def _get_nc(reps=1, variant="base", loop_reps=1):
    key = f"nc{reps}_{variant}_{loop_reps}"
    if key not in _CACHE:
        _CACHE[key] = _build_nc(reps, variant, loop_reps)
    return _CACHE[key]


def _pack_inputs(x, Wih0, Whh0, bih0, bhh0, Wih, Whh, bih, bhh, W, b):
    import ml_dtypes
    f = np.float32
    bf = ml_dtypes.bfloat16
    whhT_all = np.concatenate([Whh0[None], Whh], 0).transpose(0, 2, 1)  # (8,128,128)
    wihT_all = Wih.transpose(0, 2, 1)                                   # (7,128,128)
    biasc = np.concatenate([(bih0 + bhh0)[None], bih + bhh], 0)         # (8,128)
    wpack = np.ascontiguousarray(np.concatenate([
        whhT_all.transpose(1, 0, 2).reshape(128, 1024),
        wihT_all.transpose(1, 0, 2).reshape(128, 896),
    ], axis=1), dtype=bf)                                               # (128,1920)
    fpack = np.ascontiguousarray(
        np.concatenate([biasc.T, W], axis=1), dtype=f)                  # (128,18)
    srow = np.ascontiguousarray(Wih0[:, 0].reshape(1, 128), dtype=bf)
    frow = np.ascontiguousarray(
        np.concatenate([b, np.ones(32, f)]).reshape(1, 42), dtype=f)
    x2 = np.asarray(x, f)[:, :, 0]                                      # (256,784)
    xrows = [np.ascontiguousarray(x2[c * 32:(c + 1) * 32].T.reshape(1, 25088),
                                  dtype=bf)
             for c in range(8)]
    return {"wpack": wpack, "fpack": fpack, "srow": srow, "frow": frow}, xrows


def kernel(x, Wih0, Whh0, bih0, bhh0, Wih, Whh, bih, bhh, W, b,
           _trace=False, _trace_kwargs=None):
    _ensure_path()
    nc = _get_nc()
    from concourse.bass_utils import run_bass_kernel_spmd

    packs, xrows = _pack_inputs(
        np.asarray(x, np.float32), np.asarray(Wih0, np.float32),
        np.asarray(Whh0, np.float32), np.asarray(bih0, np.float32),
        np.asarray(bhh0, np.float32), np.asarray(Wih, np.float32),
        np.asarray(Whh, np.float32), np.asarray(bih, np.float32),
        np.asarray(bhh, np.float32), np.asarray(W, np.float32),
        np.asarray(b, np.float32))
    in_maps = [{**packs, "xrow": xrows[c]} for c in range(8)]
    kw = {}
    if _trace:
        kw["trace"] = True
        kw.update(_trace_kwargs or {})
    res = run_bass_kernel_spmd(nc, in_maps, list(range(8)), **kw)
    out = np.concatenate([np.asarray(res.results[c]["out"]) for c in range(8)], 0)
    if _trace:
        _CACHE["last_result"] = res
    return out.astype(np.float32)


# revision 32
# speedup vs baseline: 1.0212x; 1.0212x over previous
"""Trainium2 Bass kernel for the 8-layer dilated tanh-RNN stack.

Reductions used:

1) Cone reduction (exact): the model output is `h_last @ W + b` where h_last
   is the last timestep of the last layer only.  Recursively, each dilated
   layer only needs ONE of its dilation lanes, so each layer is a plain
   32-wide (batch-sharded) tanh-RNN over a subsampled sequence:
   784+392+196+98+49+25+13+7 = 1564 sequential cell steps per core.
   (Verified exact vs reference: rel err 3e-7.)

2) Chunked scan with warmup (approximate, error well below bf16 noise): the
   recurrence is contractive (|Whh| spectral radius ~0.57), so state forgets
   initial conditions at ~0.5^k.  Layers 0-3 are split into C=7 time chunks
   run in parallel as extra batch columns; each chunk (except the first)
   starts W=6 steps early from zero state to converge to the true state.
   Serial depth: L0 784 -> 118 steps of width 224.  Chunk 0's state is
   re-zeroed at the warmup/real boundary so it remains exact.

3) bf16 weights/states (PSUM accumulation, biases and classifier head stay
   fp32): measured rel err vs fp32 reference ~4e-3.

Sharding: data-parallel over batch, 32 batch elements per core, weights
replicated, no collectives.  Host packs weights transposed (lhsT) and x
time-major.
"""

import numpy as np

_CACHE: dict = {}

N_STEPS = [784, 392, 196, 98, 49, 25, 13, 7]
C = 7            # chunks for layers 0-3
W = 6            # warmup steps
M = [112, 56, 28, 14]  # chunk length (own steps) for layers 0-3


def _ensure_path():
    import sys
    for p in ("/opt/trn_rl_repo",):
        if p not in sys.path:
            sys.path.insert(0, p)


def _build_nc(reps=1, variant="base", loop_reps=1):
    _ensure_path()
    import contextlib
    import concourse.bacc as bacc
    import concourse.bass as bass
    import concourse.mybir as mybir
    import concourse.tile as tile

    f32 = mybir.dt.float32
    bf16 = mybir.dt.bfloat16
    Tanh = mybir.ActivationFunctionType.Tanh

    nc = bacc.Bacc("TRN2", target_bir_lowering=False, debug=False, num_devices=8)

    xrow_d = nc.declare_dram_parameter("xrow", (1, 25088), bf16, isOutput=False)
    wpack_d = nc.declare_dram_parameter("wpack", (128, 1920), bf16, isOutput=False)
    fpack_d = nc.declare_dram_parameter("fpack", (128, 18), f32, isOutput=False)
    srow_d = nc.declare_dram_parameter("srow", (1, 128), bf16, isOutput=False)
    frow_d = nc.declare_dram_parameter("frow", (1, 42), f32, isOutput=False)
    out_d = nc.declare_dram_parameter("out", (32, 10), f32, isOutput=True)

    with tile.TileContext(nc) as tc:
        with (
            tc.tile_pool(name="sb", bufs=1) as sb,
            tc.tile_pool(name="ps", bufs=1, space=bass.MemorySpace.PSUM) as ps,
        ):
            wpack = sb.tile([128, 1920], bf16)
            fpack = sb.tile([128, 18], f32)
            srow = sb.tile([1, 128], bf16)
            frow = sb.tile([1, 42], f32)
            # x padded row: W zero slots | 784 data slots | M[0] tail pad
            xpad = sb.tile([1, (W + 784 + M[0]) * 32], bf16)
            # chunked-consumer input buffers: W zero prefix + n slots + tail pad
            Ibuf = [None,
                    sb.tile([128, (W + 392 + M[1]) * 32], bf16, name="I1"),
                    sb.tile([128, (W + 196 + M[2]) * 32], bf16, name="I2"),
                    sb.tile([128, (W + 98 + M[3]) * 32], bf16, name="I3"),
                    sb.tile([128, (49 + C) * 32], bf16, name="I4"),
                    sb.tile([128, 512], bf16, name="I5"),
                    sb.tile([128, 512], bf16, name="I6"),
                    sb.tile([128, 512], bf16, name="I7")]
            S = ([[sb.tile([128, 224], bf16, name=f"cs{l}_{i}") for i in range(2)]
                  for l in range(4)]
                 + [sb.tile([128, 32], bf16, name=f"s{l}") for l in range(4, 8)])
            H = sb.tile([128, 32], f32)
            out_sb = sb.tile([32, 10], f32)

            # zero prefixes (read by chunk-0 warmups)
            nc.gpsimd.memset(xpad[:, 0:W * 32], 0.0)
            for l in (1, 2, 3):
                nc.gpsimd.memset(Ibuf[l][:, 0:W * 32], 0.0)

            # spread input DMAs across DGE queues so they land in parallel
            # (chunked L0's first step reads ALL of x, so startup latency is
            # gated by the slowest of these)
            nc.sync.dma_start(wpack[:], wpack_d[:])
            nc.scalar.dma_start(fpack[:], fpack_d[:])
            nc.scalar.dma_start(srow[:], srow_d[:])
            nc.scalar.dma_start(frow[:], frow_d[:])
            qs = [nc.sync, nc.scalar, nc.gpsimd]
            for c in range(8):
                qs[c % 3].dma_start(
                    xpad[:, W * 32 + c * 3136: W * 32 + (c + 1) * 3136],
                    xrow_d[:, c * 3136:(c + 1) * 3136])

            whhT = [wpack[:, l * 128:(l + 1) * 128] for l in range(8)]
            wihT = [None] + [wpack[:, 1024 + (l - 1) * 128: 1024 + l * 128]
                             for l in range(1, 8)]
            bias = [fpack[:, l: l + 1] for l in range(8)]
            wcls = fpack[:, 8:18]
            wih0T = srow[:, 0:128]
            bcls = frow[:, 0:10]
            ones32 = frow[:, 10:42]

            # PSUM: L0/L1 two banks each (4-deep slice rotation -- more WAR
            # slack on the hottest chains), L2/L3 one bank each (2-deep),
            # L4 one bank (16-slot), shared bank for L5-7 + head. Total = 8.
            cbank = [[ps.tile([128, 512], f32, name=f"cb{l}_{i}")
                      for i in range(2)] for l in range(2)] + \
                    [ps.tile([128, 512], f32, name=f"cb{l}") for l in (2, 3)]
            p4 = ps.tile([128, 512], f32, name="p4")
            shared = ps.tile([128, 512], f32, name="p567")

            def strided(buf, start_col, stride_cols):
                """[128, C, 32] AP over `buf`: cols start + k*stride for k<C."""
                sl = buf[:, start_col: start_col + C * stride_cols]
                return sl.rearrange("p (c w) -> p c w", c=C)[:, :, 0:32]

            def xstrided(start_col, stride_cols):
                sl = xpad[:, start_col: start_col + C * stride_cols]
                return sl.rearrange("p (c w) -> p c w", c=C)[:, :, 0:32]

            def psum_slice(l, s):
                if l <= 1:
                    b = cbank[l][(s % 4) // 2]
                    return b[:, (s % 2) * 256:(s % 2) * 256 + 224]
                if l <= 3:
                    return cbank[l][:, (s % 2) * 256:(s % 2) * 256 + 224]
                if l == 4:
                    return p4[:, (s % 16) * 32:(s % 16) * 32 + 32]
                if l == 5:
                    return shared[:, (s % 8) * 32:(s % 8) * 32 + 32]
                if l == 6:
                    return shared[:, 256 + (s % 4) * 32: 256 + (s % 4) * 32 + 32]
                return shared[:, 384 + (s % 2) * 32: 384 + (s % 2) * 32 + 32]

            def act_target(l, j):
                """Output AP for layer l at global step j (chunked l<=3)."""
                if l <= 3:
                    if j < W or (j - W) % 2 == 0:
                        return S[l][j % 2][:]
                    q = (j - W - 1) // 2
                    if l < 3:
                        # into Ibuf[l+1]: slot q_k = k*(M[l]/2)+q, +W prefix
                        return strided(Ibuf[l + 1], (q + W) * 32,
                                       (M[l] // 2) * 32)
                    # l == 3 -> I4 (unchunked consumer, no prefix)
                    return strided(Ibuf[4], q * 32, (M[3] // 2) * 32)
                if l <= 6:
                    if j % 2 == 0:
                        k = (j // 2) % 16
                        return Ibuf[l + 1][:, k * 32:k * 32 + 32]
                    return S[l][:]
                return H[:] if j == 6 else S[7][:]

            Tanh_ = Tanh

            def emit_act(tgt, sl, l):
                if variant == "dvecopy":
                    nc.vector.tensor_copy(tgt, sl)
                else:
                    nc.scalar.activation(tgt, sl, Tanh_, bias=bias[l])

            n_glob = [M[0] + W, M[1] + W, M[2] + W, M[3] + W, 49, 25, 13, 7]

            def can_run(l, j, progress):
                if j >= n_glob[l]:
                    return False
                if l == 0:
                    return True
                if l <= 3:
                    need = 0
                    for k in range(C):
                        s_k = k * (N_STEPS[l] // C) - W + j
                        if s_k < 0:
                            continue
                        t = 2 * s_k + 1
                        kp, loc = divmod(t, M[l - 1])
                        need = max(need, W + loc + 1)
                    return progress[l - 1] >= need
                if l == 4:
                    t = 2 * j + 1
                    kp, loc = divmod(t, M[3])
                    return progress[3] >= W + loc + 1
                return progress[l - 1] >= 2 * j + 1

            def emit_step(l, j, prev):
                if l <= 3:
                    sl = psum_slice(l, j)
                    if l == 0:
                        nc.tensor.matmul(sl, wih0T, xstrided(j * 32, M[0] * 32),
                                         start=True, stop=(j == 0),
                                         skip_group_check=True)
                    else:
                        rhs = strided(Ibuf[l], j * 32, (N_STEPS[l] // C) * 32)
                        nc.tensor.matmul(sl, wihT[l], rhs,
                                         start=True, stop=(j == 0),
                                         skip_group_check=True)
                    if j > 0:
                        nc.tensor.matmul(sl, whhT[l], prev[l],
                                         start=False, stop=True,
                                         skip_group_check=True)
                    tgt = act_target(l, j)
                    emit_act(tgt, sl, l)
                    prev[l] = tgt
                    if j == W - 1:
                        # chunk 0 enters its real region from exact zero state
                        # (tgt at j=W-1 is S[l][(W-1)%2]; zero chunk 0's cols)
                        nc.gpsimd.memset(S[l][(W - 1) % 2][:, 0:32], 0.0)
                else:
                    sl = psum_slice(l, j)
                    k = j % 16
                    nc.tensor.matmul(sl, wihT[l], Ibuf[l][:, j * 32:(j + 1) * 32]
                                     if l == 4 else Ibuf[l][:, k * 32:k * 32 + 32],
                                     start=True, stop=(j == 0),
                                     skip_group_check=True)
                    if j > 0:
                        nc.tensor.matmul(sl, whhT[l], prev[l],
                                         start=False, stop=True,
                                         skip_group_check=True)
                    tgt = act_target(l, j)
                    emit_act(tgt, sl, l)
                    prev[l] = tgt

            def emit_cone():
                prev = [None] * 8
                progress = [0] * 8
                total = sum(n_glob)
                done = 0
                while done < total:
                    stuck = True
                    for l in range(8):
                        if can_run(l, progress[l], progress):
                            emit_step(l, progress[l], prev)
                            progress[l] += 1
                            done += 1
                            stuck = False
                    assert not stuck, f"schedule deadlock at {progress}"
                # classifier head
                head = shared[0:32, 448:458]
                nc.tensor.matmul(head, H[:], wcls, start=True, stop=False,
                                 skip_group_check=True)
                nc.tensor.matmul(head, ones32, bcls, start=False, stop=True,
                                 skip_group_check=True)
                nc.vector.tensor_copy(out_sb[:], head)

            loop_ctx = (tc.For_i(0, loop_reps, 1) if loop_reps > 1
                        else contextlib.nullcontext())
            with loop_ctx:
                for _rep in range(reps):
                    emit_cone()

            nc.sync.dma_start(out_d[:], out_sb[:])

    nc.compile()
    return nc


def _get_nc(reps=1, variant="base", loop_reps=1):
    key = f"nc{reps}_{variant}_{loop_reps}"
    if key not in _CACHE:
        _CACHE[key] = _build_nc(reps, variant, loop_reps)
    return _CACHE[key]


def _pack_inputs(x, Wih0, Whh0, bih0, bhh0, Wih, Whh, bih, bhh, W, b):
    import ml_dtypes
    f = np.float32
    bf = ml_dtypes.bfloat16
    whhT_all = np.concatenate([Whh0[None], Whh], 0).transpose(0, 2, 1)  # (8,128,128)
    wihT_all = Wih.transpose(0, 2, 1)                                   # (7,128,128)
    biasc = np.concatenate([(bih0 + bhh0)[None], bih + bhh], 0)         # (8,128)
    wpack = np.ascontiguousarray(np.concatenate([
        whhT_all.transpose(1, 0, 2).reshape(128, 1024),
        wihT_all.transpose(1, 0, 2).reshape(128, 896),
    ], axis=1), dtype=bf)                                               # (128,1920)
    fpack = np.ascontiguousarray(
        np.concatenate([biasc.T, W], axis=1), dtype=f)                  # (128,18)
    srow = np.ascontiguousarray(Wih0[:, 0].reshape(1, 128), dtype=bf)
    frow = np.ascontiguousarray(
        np.concatenate([b, np.ones(32, f)]).reshape(1, 42), dtype=f)
    x2 = np.asarray(x, f)[:, :, 0]                                      # (256,784)
    xrows = [np.ascontiguousarray(x2[c * 32:(c + 1) * 32].T.reshape(1, 25088),
                                  dtype=bf)
             for c in range(8)]
    return {"wpack": wpack, "fpack": fpack, "srow": srow, "frow": frow}, xrows


def kernel(x, Wih0, Whh0, bih0, bhh0, Wih, Whh, bih, bhh, W, b,
           _trace=False, _trace_kwargs=None):
    _ensure_path()
    nc = _get_nc()
    from concourse.bass_utils import run_bass_kernel_spmd

    packs, xrows = _pack_inputs(
        np.asarray(x, np.float32), np.asarray(Wih0, np.float32),
        np.asarray(Whh0, np.float32), np.asarray(bih0, np.float32),
        np.asarray(bhh0, np.float32), np.asarray(Wih, np.float32),
        np.asarray(Whh, np.float32), np.asarray(bih, np.float32),
        np.asarray(bhh, np.float32), np.asarray(W, np.float32),
        np.asarray(b, np.float32))
    in_maps = [{**packs, "xrow": xrows[c]} for c in range(8)]
    kw = {}
    if _trace:
        kw["trace"] = True
        kw.update(_trace_kwargs or {})
    res = run_bass_kernel_spmd(nc, in_maps, list(range(8)), **kw)
    out = np.concatenate([np.asarray(res.results[c]["out"]) for c in range(8)], 0)
    if _trace:
        _CACHE["last_result"] = res
    return out.astype(np.float32)


# revision 34
# speedup vs baseline: 1.0448x; 1.0231x over previous
"""Trainium2 Bass kernel for the 8-layer dilated tanh-RNN stack.

Reductions used:

1) Cone reduction (exact): the model output is `h_last @ W + b` where h_last
   is the last timestep of the last layer only.  Recursively, each dilated
   layer only needs ONE of its dilation lanes, so each layer is a plain
   32-wide (batch-sharded) tanh-RNN over a subsampled sequence:
   784+392+196+98+49+25+13+7 = 1564 sequential cell steps per core.
   (Verified exact vs reference: rel err 3e-7.)

2) Chunked scan with warmup (approximate, error well below bf16 noise): the
   recurrence is contractive (|Whh| spectral radius ~0.57), so state forgets
   initial conditions at ~0.5^k.  Layers 0-3 are split into C=7 time chunks
   run in parallel as extra batch columns; each chunk (except the first)
   starts W=4 steps early from zero state to converge to the true state.
   Serial depth: L0 784 -> 116 steps of width 224.  Chunk 0's state is
   re-zeroed at the warmup/real boundary so it remains exact.

3) bf16 weights/states (PSUM accumulation, biases and classifier head stay
   fp32): measured rel err vs fp32 reference ~4e-3.

Sharding: data-parallel over batch, 32 batch elements per core, weights
replicated, no collectives.  Host packs weights transposed (lhsT) and x
time-major.
"""

import numpy as np

_CACHE: dict = {}

N_STEPS = [784, 392, 196, 98, 49, 25, 13, 7]
C = 7            # chunks for layers 0-3
W = 4            # warmup steps
M = [112, 56, 28, 14]  # chunk length (own steps) for layers 0-3


def _ensure_path():
    import sys
    for p in ("/opt/trn_rl_repo",):
        if p not in sys.path:
            sys.path.insert(0, p)


def _build_nc(reps=1, variant="base", loop_reps=1):
    _ensure_path()
    import contextlib
    import concourse.bacc as bacc
    import concourse.bass as bass
    import concourse.mybir as mybir
    import concourse.tile as tile

    f32 = mybir.dt.float32
    bf16 = mybir.dt.bfloat16
    Tanh = mybir.ActivationFunctionType.Tanh

    nc = bacc.Bacc("TRN2", target_bir_lowering=False, debug=False, num_devices=8)

    xrow_d = nc.declare_dram_parameter("xrow", (1, 25088), bf16, isOutput=False)
    wpack_d = nc.declare_dram_parameter("wpack", (128, 1920), bf16, isOutput=False)
    fpack_d = nc.declare_dram_parameter("fpack", (128, 18), f32, isOutput=False)
    srow_d = nc.declare_dram_parameter("srow", (1, 128), bf16, isOutput=False)
    frow_d = nc.declare_dram_parameter("frow", (1, 42), f32, isOutput=False)
    out_d = nc.declare_dram_parameter("out", (32, 10), f32, isOutput=True)

    with tile.TileContext(nc) as tc:
        with (
            tc.tile_pool(name="sb", bufs=1) as sb,
            tc.tile_pool(name="ps", bufs=1, space=bass.MemorySpace.PSUM) as ps,
        ):
            wpack = sb.tile([128, 1920], bf16)
            fpack = sb.tile([128, 18], f32)
            srow = sb.tile([1, 128], bf16)
            frow = sb.tile([1, 42], f32)
            # x padded row: W zero slots | 784 data slots | M[0] tail pad
            xpad = sb.tile([1, (W + 784 + M[0]) * 32], bf16)
            # chunked-consumer input buffers: W zero prefix + n slots + tail pad
            Ibuf = [None,
                    sb.tile([128, (W + 392 + M[1]) * 32], bf16, name="I1"),
                    sb.tile([128, (W + 196 + M[2]) * 32], bf16, name="I2"),
                    sb.tile([128, (W + 98 + M[3]) * 32], bf16, name="I3"),
                    sb.tile([128, (49 + C) * 32], bf16, name="I4"),
                    sb.tile([128, 512], bf16, name="I5"),
                    sb.tile([128, 512], bf16, name="I6"),
                    sb.tile([128, 512], bf16, name="I7")]
            S = ([[sb.tile([128, 224], bf16, name=f"cs{l}_{i}") for i in range(2)]
                  for l in range(4)]
                 + [sb.tile([128, 32], bf16, name=f"s{l}") for l in range(4, 8)])
            H = sb.tile([128, 32], f32)
            out_sb = sb.tile([32, 10], f32)

            # zero prefixes (read by chunk-0 warmups)
            nc.gpsimd.memset(xpad[:, 0:W * 32], 0.0)
            for l in (1, 2, 3):
                nc.gpsimd.memset(Ibuf[l][:, 0:W * 32], 0.0)

            # spread input DMAs across DGE queues so they land in parallel
            # (chunked L0's first step reads ALL of x, so startup latency is
            # gated by the slowest of these)
            nc.sync.dma_start(wpack[:], wpack_d[:])
            nc.scalar.dma_start(fpack[:], fpack_d[:])
            nc.scalar.dma_start(srow[:], srow_d[:])
            nc.scalar.dma_start(frow[:], frow_d[:])
            qs = [nc.sync, nc.scalar, nc.gpsimd]
            for c in range(8):
                qs[c % 3].dma_start(
                    xpad[:, W * 32 + c * 3136: W * 32 + (c + 1) * 3136],
                    xrow_d[:, c * 3136:(c + 1) * 3136])

            whhT = [wpack[:, l * 128:(l + 1) * 128] for l in range(8)]
            wihT = [None] + [wpack[:, 1024 + (l - 1) * 128: 1024 + l * 128]
                             for l in range(1, 8)]
            bias = [fpack[:, l: l + 1] for l in range(8)]
            wcls = fpack[:, 8:18]
            wih0T = srow[:, 0:128]
            bcls = frow[:, 0:10]
            ones32 = frow[:, 10:42]

            # PSUM: L0/L1 two banks each (4-deep slice rotation -- more WAR
            # slack on the hottest chains), L2/L3 one bank each (2-deep),
            # L4 one bank (16-slot), shared bank for L5-7 + head. Total = 8.
            cbank = [[ps.tile([128, 512], f32, name=f"cb{l}_{i}")
                      for i in range(2)] for l in range(2)] + \
                    [ps.tile([128, 512], f32, name=f"cb{l}") for l in (2, 3)]
            p4 = ps.tile([128, 512], f32, name="p4")
            shared = ps.tile([128, 512], f32, name="p567")

            def strided(buf, start_col, stride_cols):
                """[128, C, 32] AP over `buf`: cols start + k*stride for k<C."""
                sl = buf[:, start_col: start_col + C * stride_cols]
                return sl.rearrange("p (c w) -> p c w", c=C)[:, :, 0:32]

            def xstrided(start_col, stride_cols):
                sl = xpad[:, start_col: start_col + C * stride_cols]
                return sl.rearrange("p (c w) -> p c w", c=C)[:, :, 0:32]

            def psum_slice(l, s):
                if l <= 1:
                    b = cbank[l][(s % 4) // 2]
                    return b[:, (s % 2) * 256:(s % 2) * 256 + 224]
                if l <= 3:
                    return cbank[l][:, (s % 2) * 256:(s % 2) * 256 + 224]
                if l == 4:
                    return p4[:, (s % 16) * 32:(s % 16) * 32 + 32]
                if l == 5:
                    return shared[:, (s % 8) * 32:(s % 8) * 32 + 32]
                if l == 6:
                    return shared[:, 256 + (s % 4) * 32: 256 + (s % 4) * 32 + 32]
                return shared[:, 384 + (s % 2) * 32: 384 + (s % 2) * 32 + 32]

            def act_target(l, j):
                """Output AP for layer l at global step j (chunked l<=3)."""
                if l <= 3:
                    if j < W or (j - W) % 2 == 0:
                        return S[l][j % 2][:]
                    q = (j - W - 1) // 2
                    if l < 3:
                        # into Ibuf[l+1]: slot q_k = k*(M[l]/2)+q, +W prefix
                        return strided(Ibuf[l + 1], (q + W) * 32,
                                       (M[l] // 2) * 32)
                    # l == 3 -> I4 (unchunked consumer, no prefix)
                    return strided(Ibuf[4], q * 32, (M[3] // 2) * 32)
                if l <= 6:
                    if j % 2 == 0:
                        k = (j // 2) % 16
                        return Ibuf[l + 1][:, k * 32:k * 32 + 32]
                    return S[l][:]
                return H[:] if j == 6 else S[7][:]

            Tanh_ = Tanh

            def emit_act(tgt, sl, l):
                if variant == "dvecopy":
                    nc.vector.tensor_copy(tgt, sl)
                else:
                    nc.scalar.activation(tgt, sl, Tanh_, bias=bias[l])

            n_glob = [M[0] + W, M[1] + W, M[2] + W, M[3] + W, 49, 25, 13, 7]

            def can_run(l, j, progress):
                if j >= n_glob[l]:
                    return False
                if l == 0:
                    return True
                if l <= 3:
                    need = 0
                    for k in range(C):
                        s_k = k * (N_STEPS[l] // C) - W + j
                        if s_k < 0:
                            continue
                        t = 2 * s_k + 1
                        kp, loc = divmod(t, M[l - 1])
                        need = max(need, W + loc + 1)
                    return progress[l - 1] >= need
                if l == 4:
                    t = 2 * j + 1
                    kp, loc = divmod(t, M[3])
                    return progress[3] >= W + loc + 1
                return progress[l - 1] >= 2 * j + 1

            def emit_step(l, j, prev):
                if l <= 3:
                    sl = psum_slice(l, j)
                    if l == 0:
                        nc.tensor.matmul(sl, wih0T, xstrided(j * 32, M[0] * 32),
                                         start=True, stop=(j == 0),
                                         skip_group_check=True)
                    else:
                        rhs = strided(Ibuf[l], j * 32, (N_STEPS[l] // C) * 32)
                        nc.tensor.matmul(sl, wihT[l], rhs,
                                         start=True, stop=(j == 0),
                                         skip_group_check=True)
                    if j > 0:
                        nc.tensor.matmul(sl, whhT[l], prev[l],
                                         start=False, stop=True,
                                         skip_group_check=True)
                    tgt = act_target(l, j)
                    emit_act(tgt, sl, l)
                    prev[l] = tgt
                    if j == W - 1:
                        # chunk 0 enters its real region from exact zero state
                        # (tgt at j=W-1 is S[l][(W-1)%2]; zero chunk 0's cols)
                        nc.gpsimd.memset(S[l][(W - 1) % 2][:, 0:32], 0.0)
                else:
                    sl = psum_slice(l, j)
                    k = j % 16
                    nc.tensor.matmul(sl, wihT[l], Ibuf[l][:, j * 32:(j + 1) * 32]
                                     if l == 4 else Ibuf[l][:, k * 32:k * 32 + 32],
                                     start=True, stop=(j == 0),
                                     skip_group_check=True)
                    if j > 0:
                        nc.tensor.matmul(sl, whhT[l], prev[l],
                                         start=False, stop=True,
                                         skip_group_check=True)
                    tgt = act_target(l, j)
                    emit_act(tgt, sl, l)
                    prev[l] = tgt

            def emit_cone():
                prev = [None] * 8
                progress = [0] * 8
                total = sum(n_glob)
                done = 0
                while done < total:
                    stuck = True
                    for l in range(8):
                        if can_run(l, progress[l], progress):
                            emit_step(l, progress[l], prev)
                            progress[l] += 1
                            done += 1
                            stuck = False
                    assert not stuck, f"schedule deadlock at {progress}"
                # classifier head
                head = shared[0:32, 448:458]
                nc.tensor.matmul(head, H[:], wcls, start=True, stop=False,
                                 skip_group_check=True)
                nc.tensor.matmul(head, ones32, bcls, start=False, stop=True,
                                 skip_group_check=True)
                nc.vector.tensor_copy(out_sb[:], head)

            loop_ctx = (tc.For_i(0, loop_reps, 1) if loop_reps > 1
                        else contextlib.nullcontext())
            with loop_ctx:
                for _rep in range(reps):
                    emit_cone()

            nc.sync.dma_start(out_d[:], out_sb[:])

    nc.compile()
    return nc


def _get_nc(reps=1, variant="base", loop_reps=1):
    key = f"nc{reps}_{variant}_{loop_reps}"
    if key not in _CACHE:
        _CACHE[key] = _build_nc(reps, variant, loop_reps)
    return _CACHE[key]


def _pack_inputs(x, Wih0, Whh0, bih0, bhh0, Wih, Whh, bih, bhh, W, b):
    import ml_dtypes
    f = np.float32
    bf = ml_dtypes.bfloat16
    whhT_all = np.concatenate([Whh0[None], Whh], 0).transpose(0, 2, 1)  # (8,128,128)
    wihT_all = Wih.transpose(0, 2, 1)                                   # (7,128,128)
    biasc = np.concatenate([(bih0 + bhh0)[None], bih + bhh], 0)         # (8,128)
    wpack = np.ascontiguousarray(np.concatenate([
        whhT_all.transpose(1, 0, 2).reshape(128, 1024),
        wihT_all.transpose(1, 0, 2).reshape(128, 896),
    ], axis=1), dtype=bf)                                               # (128,1920)
    fpack = np.ascontiguousarray(
        np.concatenate([biasc.T, W], axis=1), dtype=f)                  # (128,18)
    srow = np.ascontiguousarray(Wih0[:, 0].reshape(1, 128), dtype=bf)
    frow = np.ascontiguousarray(
        np.concatenate([b, np.ones(32, f)]).reshape(1, 42), dtype=f)
    x2 = np.asarray(x, f)[:, :, 0]                                      # (256,784)
    xrows = [np.ascontiguousarray(x2[c * 32:(c + 1) * 32].T.reshape(1, 25088),
                                  dtype=bf)
             for c in range(8)]
    return {"wpack": wpack, "fpack": fpack, "srow": srow, "frow": frow}, xrows


def kernel(x, Wih0, Whh0, bih0, bhh0, Wih, Whh, bih, bhh, W, b,
           _trace=False, _trace_kwargs=None):
    _ensure_path()
    nc = _get_nc()
    from concourse.bass_utils import run_bass_kernel_spmd

    packs, xrows = _pack_inputs(
        np.asarray(x, np.float32), np.asarray(Wih0, np.float32),
        np.asarray(Whh0, np.float32), np.asarray(bih0, np.float32),
        np.asarray(bhh0, np.float32), np.asarray(Wih, np.float32),
        np.asarray(Whh, np.float32), np.asarray(bih, np.float32),
        np.asarray(bhh, np.float32), np.asarray(W, np.float32),
        np.asarray(b, np.float32))
    in_maps = [{**packs, "xrow": xrows[c]} for c in range(8)]
    kw = {}
    if _trace:
        kw["trace"] = True
        kw.update(_trace_kwargs or {})
    res = run_bass_kernel_spmd(nc, in_maps, list(range(8)), **kw)
    out = np.concatenate([np.asarray(res.results[c]["out"]) for c in range(8)], 0)
    if _trace:
        _CACHE["last_result"] = res
    return out.astype(np.float32)


# revision 36
# speedup vs baseline: 1.0682x; 1.0223x over previous
"""Trainium2 Bass kernel for the 8-layer dilated tanh-RNN stack.

Reductions used:

1) Cone reduction (exact): the model output is `h_last @ W + b` where h_last
   is the last timestep of the last layer only.  Recursively, each dilated
   layer only needs ONE of its dilation lanes, so each layer is a plain
   32-wide (batch-sharded) tanh-RNN over a subsampled sequence:
   784+392+196+98+49+25+13+7 = 1564 sequential cell steps per core.
   (Verified exact vs reference: rel err 3e-7.)

2) Chunked scan with warmup (approximate, error well below bf16 noise): the
   recurrence is contractive (|Whh| spectral radius ~0.57), so state forgets
   initial conditions at ~0.5^k.  Layers 0-3 are split into C=7 time chunks
   run in parallel as extra batch columns; each chunk (except the first)
   starts W=4 steps early from zero state to converge to the true state.
   Serial depth: L0 784 -> 116 steps of width 224.  Chunk 0's state is
   re-zeroed at the warmup/real boundary so it remains exact.

3) bf16 weights/states (PSUM accumulation, biases and classifier head stay
   fp32): measured rel err vs fp32 reference ~4e-3.

Sharding: data-parallel over batch, 32 batch elements per core, weights
replicated, no collectives.  Host packs weights transposed (lhsT) and x
time-major.
"""

import numpy as np

_CACHE: dict = {}

N_STEPS = [784, 392, 196, 98, 49, 25, 13, 7]
C = 7            # chunks for layers 0-3
W = 4            # warmup steps
M = [112, 56, 28, 14]  # chunk length (own steps) for layers 0-3


def _ensure_path():
    import sys
    for p in ("/opt/trn_rl_repo",):
        if p not in sys.path:
            sys.path.insert(0, p)


def _build_nc(reps=1, variant="base", loop_reps=1):
    _ensure_path()
    import contextlib
    import concourse.bacc as bacc
    import concourse.bass as bass
    import concourse.mybir as mybir
    import concourse.tile as tile

    f32 = mybir.dt.float32
    bf16 = mybir.dt.bfloat16
    Tanh = mybir.ActivationFunctionType.Tanh

    nc = bacc.Bacc("TRN2", target_bir_lowering=False, debug=False, num_devices=8)

    xrow_d = nc.declare_dram_parameter("xrow", (1, 25088), bf16, isOutput=False)
    wpack_d = nc.declare_dram_parameter("wpack", (128, 1920), bf16, isOutput=False)
    fpack_d = nc.declare_dram_parameter("fpack", (128, 18), f32, isOutput=False)
    srow_d = nc.declare_dram_parameter("srow", (1, 128), bf16, isOutput=False)
    frow_d = nc.declare_dram_parameter("frow", (1, 42), f32, isOutput=False)
    out_d = nc.declare_dram_parameter("out", (32, 10), f32, isOutput=True)

    with tile.TileContext(nc) as tc:
        with (
            tc.tile_pool(name="sb", bufs=1) as sb,
            tc.tile_pool(name="ps", bufs=1, space=bass.MemorySpace.PSUM) as ps,
        ):
            wpack = sb.tile([128, 1920], bf16)
            fpack = sb.tile([128, 18], f32)
            srow = sb.tile([1, 128], bf16)
            frow = sb.tile([1, 42], f32)
            # x padded row: W zero slots | 784 data slots | M[0] tail pad
            xpad = sb.tile([1, (W + 784 + M[0]) * 32], bf16)
            # chunked-consumer input buffers: W zero prefix + n slots + tail pad
            Ibuf = [None,
                    sb.tile([128, (W + 392 + M[1]) * 32], bf16, name="I1"),
                    sb.tile([128, (W + 196 + M[2]) * 32], bf16, name="I2"),
                    sb.tile([128, (W + 98 + M[3]) * 32], bf16, name="I3"),
                    sb.tile([128, (49 + C) * 32], bf16, name="I4"),
                    sb.tile([128, 512], bf16, name="I5"),
                    sb.tile([128, 512], bf16, name="I6"),
                    sb.tile([128, 512], bf16, name="I7")]
            S = ([[sb.tile([128, 224], bf16, name=f"cs{l}_{i}") for i in range(2)]
                  for l in range(4)]
                 + [sb.tile([128, 32], bf16, name=f"s{l}") for l in range(4, 8)])
            H = sb.tile([128, 32], f32)
            out_sb = sb.tile([32, 10], f32)

            # zero prefixes (read by chunk-0 warmups)
            nc.gpsimd.memset(xpad[:, 0:W * 32], 0.0)
            for l in (1, 2, 3):
                nc.gpsimd.memset(Ibuf[l][:, 0:W * 32], 0.0)

            # spread input DMAs across DGE queues so they land in parallel
            # (chunked L0's first step reads ALL of x, so startup latency is
            # gated by the slowest of these)
            nc.sync.dma_start(wpack[:], wpack_d[:])
            nc.scalar.dma_start(fpack[:], fpack_d[:])
            nc.scalar.dma_start(srow[:], srow_d[:])
            nc.scalar.dma_start(frow[:], frow_d[:])
            qs = [nc.sync, nc.scalar, nc.gpsimd]
            for c in range(8):
                qs[c % 3].dma_start(
                    xpad[:, W * 32 + c * 3136: W * 32 + (c + 1) * 3136],
                    xrow_d[:, c * 3136:(c + 1) * 3136])

            whhT = [wpack[:, l * 128:(l + 1) * 128] for l in range(8)]
            wihT = [None] + [wpack[:, 1024 + (l - 1) * 128: 1024 + l * 128]
                             for l in range(1, 8)]
            bias = [fpack[:, l: l + 1] for l in range(8)]
            wcls = fpack[:, 8:18]
            wih0T = srow[:, 0:128]
            bcls = frow[:, 0:10]
            ones32 = frow[:, 10:42]

            # PSUM: L0/L1 two banks each (4-deep slice rotation -- more WAR
            # slack on the hottest chains), L2/L3 one bank each (2-deep),
            # L4 one bank (16-slot), shared bank for L5-7 + head. Total = 8.
            cbank = [[ps.tile([128, 512], f32, name=f"cb{l}_{i}")
                      for i in range(2)] for l in range(2)] + \
                    [ps.tile([128, 512], f32, name=f"cb{l}") for l in (2, 3)]
            p4 = ps.tile([128, 512], f32, name="p4")
            shared = ps.tile([128, 512], f32, name="p567")

            def strided(buf, start_col, stride_cols):
                """[128, C, 32] AP over `buf`: cols start + k*stride for k<C."""
                sl = buf[:, start_col: start_col + C * stride_cols]
                return sl.rearrange("p (c w) -> p c w", c=C)[:, :, 0:32]

            def xstrided(start_col, stride_cols):
                sl = xpad[:, start_col: start_col + C * stride_cols]
                return sl.rearrange("p (c w) -> p c w", c=C)[:, :, 0:32]

            def psum_slice(l, s):
                if l <= 1:
                    b = cbank[l][(s % 4) // 2]
                    return b[:, (s % 2) * 256:(s % 2) * 256 + 224]
                if l <= 3:
                    return cbank[l][:, (s % 2) * 256:(s % 2) * 256 + 224]
                if l == 4:
                    return p4[:, (s % 16) * 32:(s % 16) * 32 + 32]
                if l == 5:
                    return shared[:, (s % 8) * 32:(s % 8) * 32 + 32]
                if l == 6:
                    return shared[:, 256 + (s % 4) * 32: 256 + (s % 4) * 32 + 32]
                return shared[:, 384 + (s % 2) * 32: 384 + (s % 2) * 32 + 32]

            def act_target(l, j):
                """Output AP for layer l at global step j (chunked l<=3)."""
                if l <= 3:
                    if j < W or (j - W) % 2 == 0:
                        return S[l][j % 2][:]
                    q = (j - W - 1) // 2
                    if l < 3:
                        # into Ibuf[l+1]: slot q_k = k*(M[l]/2)+q, +W prefix
                        return strided(Ibuf[l + 1], (q + W) * 32,
                                       (M[l] // 2) * 32)
                    # l == 3 -> I4 (unchunked consumer, no prefix)
                    return strided(Ibuf[4], q * 32, (M[3] // 2) * 32)
                if l <= 6:
                    if j % 2 == 0:
                        k = (j // 2) % 16
                        return Ibuf[l + 1][:, k * 32:k * 32 + 32]
                    return S[l][:]
                return H[:] if j == 6 else S[7][:]

            Tanh_ = Tanh

            def emit_act(tgt, sl, l):
                if variant == "dvecopy":
                    nc.vector.tensor_copy(tgt, sl)
                else:
                    nc.scalar.activation(tgt, sl, Tanh_, bias=bias[l])

            n_glob = [M[0] + W, M[1] + W, M[2] + W, M[3] + W, 49, 25, 13, 7]

            def can_run(l, j, progress):
                if j >= n_glob[l]:
                    return False
                if l == 0:
                    return True
                if l <= 3:
                    need = 0
                    for k in range(C):
                        s_k = k * (N_STEPS[l] // C) - W + j
                        if s_k < 0:
                            continue
                        t = 2 * s_k + 1
                        kp, loc = divmod(t, M[l - 1])
                        need = max(need, W + loc + 1)
                    return progress[l - 1] >= need
                if l == 4:
                    t = 2 * j + 1
                    kp, loc = divmod(t, M[3])
                    return progress[3] >= W + loc + 1
                return progress[l - 1] >= 2 * j + 1

            def emit_step(l, j, prev):
                if l <= 3:
                    sl = psum_slice(l, j)
                    if l == 0:
                        nc.tensor.matmul(sl, wih0T, xstrided(j * 32, M[0] * 32),
                                         start=True, stop=(j == 0),
                                         skip_group_check=True)
                    else:
                        rhs = strided(Ibuf[l], j * 32, (N_STEPS[l] // C) * 32)
                        nc.tensor.matmul(sl, wihT[l], rhs,
                                         start=True, stop=(j == 0),
                                         skip_group_check=True)
                    if j > 0:
                        nc.tensor.matmul(sl, whhT[l], prev[l],
                                         start=False, stop=True,
                                         skip_group_check=True)
                    tgt = act_target(l, j)
                    emit_act(tgt, sl, l)
                    prev[l] = tgt
                    if j == W - 1:
                        # chunk 0 enters its real region from exact zero state
                        # (tgt at j=W-1 is S[l][(W-1)%2]; zero chunk 0's cols)
                        nc.gpsimd.memset(S[l][(W - 1) % 2][:, 0:32], 0.0)
                else:
                    sl = psum_slice(l, j)
                    k = j % 16
                    nc.tensor.matmul(sl, wihT[l], Ibuf[l][:, j * 32:(j + 1) * 32]
                                     if l == 4 else Ibuf[l][:, k * 32:k * 32 + 32],
                                     start=True, stop=(j == 0),
                                     skip_group_check=True)
                    if j > 0:
                        nc.tensor.matmul(sl, whhT[l], prev[l],
                                         start=False, stop=True,
                                         skip_group_check=True)
                    tgt = act_target(l, j)
                    emit_act(tgt, sl, l)
                    prev[l] = tgt

            def emit_cone():
                prev = [None] * 8
                progress = [0] * 8
                total = sum(n_glob)
                done = 0
                while done < total:
                    stuck = True
                    for l in range(8):
                        if can_run(l, progress[l], progress):
                            emit_step(l, progress[l], prev)
                            progress[l] += 1
                            done += 1
                            stuck = False
                    assert not stuck, f"schedule deadlock at {progress}"
                # classifier head
                head = shared[0:32, 448:458]
                nc.tensor.matmul(head, H[:], wcls, start=True, stop=False,
                                 skip_group_check=True)
                nc.tensor.matmul(head, ones32, bcls, start=False, stop=True,
                                 skip_group_check=True)
                nc.vector.tensor_copy(out_sb[:], head)

            loop_ctx = (tc.For_i(0, loop_reps, 1) if loop_reps > 1
                        else contextlib.nullcontext())
            with loop_ctx:
                for _rep in range(reps):
                    emit_cone()

            nc.sync.dma_start(out_d[:], out_sb[:])

    nc.compile()
    return nc


def _get_nc(reps=1, variant="base", loop_reps=1):
    key = f"nc{reps}_{variant}_{loop_reps}"
    if key not in _CACHE:
        _CACHE[key] = _build_nc(reps, variant, loop_reps)
    return _CACHE[key]


def _pack_inputs(x, Wih0, Whh0, bih0, bhh0, Wih, Whh, bih, bhh, W, b):
    import ml_dtypes
    f = np.float32
    bf = ml_dtypes.bfloat16
    whhT_all = np.concatenate([Whh0[None], Whh], 0).transpose(0, 2, 1)  # (8,128,128)
    wihT_all = Wih.transpose(0, 2, 1)                                   # (7,128,128)
    biasc = np.concatenate([(bih0 + bhh0)[None], bih + bhh], 0)         # (8,128)
    wpack = np.ascontiguousarray(np.concatenate([
        whhT_all.transpose(1, 0, 2).reshape(128, 1024),
        wihT_all.transpose(1, 0, 2).reshape(128, 896),
    ], axis=1), dtype=bf)                                               # (128,1920)
    fpack = np.ascontiguousarray(
        np.concatenate([biasc.T, W], axis=1), dtype=f)                  # (128,18)
    srow = np.ascontiguousarray(Wih0[:, 0].reshape(1, 128), dtype=bf)
    frow = np.ascontiguousarray(
        np.concatenate([b, np.ones(32, f)]).reshape(1, 42), dtype=f)
    x2 = np.asarray(x, f)[:, :, 0]                                      # (256,784)
    xrows = [np.ascontiguousarray(x2[c * 32:(c + 1) * 32].T.reshape(1, 25088),
                                  dtype=bf)
             for c in range(8)]
    return {"wpack": wpack, "fpack": fpack, "srow": srow, "frow": frow}, xrows


def kernel(x, Wih0, Whh0, bih0, bhh0, Wih, Whh, bih, bhh, W, b,
           _trace=False, _trace_kwargs=None):
    _ensure_path()
    nc = _get_nc()
    from concourse.bass_utils import run_bass_kernel_spmd

    packs, xrows = _pack_inputs(
        np.asarray(x, np.float32), np.asarray(Wih0, np.float32),
        np.asarray(Whh0, np.float32), np.asarray(bih0, np.float32),
        np.asarray(bhh0, np.float32), np.asarray(Wih, np.float32),
        np.asarray(Whh, np.float32), np.asarray(bih, np.float32),
        np.asarray(bhh, np.float32), np.asarray(W, np.float32),
        np.asarray(b, np.float32))
    in_maps = [{**packs, "xrow": xrows[c]} for c in range(8)]
    kw = {}
    if _trace:
        kw["trace"] = True
        kw.update(_trace_kwargs or {})
    res = run_bass_kernel_spmd(nc, in_maps, list(range(8)), **kw)
    out = np.concatenate([np.asarray(res.results[c]["out"]) for c in range(8)], 0)
    if _trace:
        _CACHE["last_result"] = res
    return out.astype(np.float32)
